# revision 4
# baseline (speedup 1.0000x reference)
"""Batch-hard triplet loss on 8 Trainium2 NeuronCores (Bass/Tile).

Strategy (data-parallel over anchor rows):
  Each core owns R = B/8 anchor rows and computes the [R, B] score block

      S[m, n] = ||e_m - e_n||^2 + C * [label_m == label_n]

  via ONE augmented matmul:  S = lhsT.T @ rhs + sq_m, with
      rhs  = [ -2*emb.T ; onehot.T ; sq_hi ; sq_lo ]   (streamed, full B cols)
      lhsT = [ emb.T_local ; C*onehot.T_local ; 1 ; 1 ] (resident)

  With C larger than any squared distance:
      hard-positive d2 = rowmax(S) - C - sq_m     (same-label entries sit at d2+C)
      hard-negative d2 = rowmin(S)     - sq_m     (diff-label entries sit at d2)
  The diagonal sits at exactly ~C: never the max when a real positive exists
  (those are C + d2 > C), never the min when a real negative exists
  (those are d2 < C).  Rows without positives/negatives are masked by `valid`.

  No argmax indices or gathers are needed anywhere — only row max/min values.
  Feature tiles use float32r (full PE rate at N=512, ~tf32 precision); the
  one-hot block and hi/lo-split sq rows are bf16 (exact for 0/1, C, and the
  split).  Loss tail (sqrt, margin, relu, valid-mask, partition-sum) runs on
  ACT/DVE/PE; the host only sums 4 partials per core and divides by the
  valid count.
"""

import numpy as np
import ml_dtypes

B = 4096
D = 2048
NCORES = 8
L = 128          # number of label classes (labels are in [0, 128))
P = 128          # partitions
NT = 512         # n-tile (matmul free dim = one PSUM bank of f32)
CBIG = 32768.0   # separation constant; must exceed max squared distance
MARGIN = 0.3

TRACE = False           # test.py sets this to profile
LAST_RESULT = None      # BassKernelResults of the most recent run

_cache = {}


def _build(b, d, n_cores, l=L, nt=NT, repeat=1):
    """Build + compile the per-core Bass kernel (same NEFF for all cores).

    repeat>1 emits the whole body N times (bench builds: slope timing)."""
    from contextlib import ExitStack

    import concourse.mybir as mybir
    import concourse.tile as tile
    from concourse import bacc

    r = b // n_cores      # local anchor rows per core
    mc = r // P           # m-chunks of 128 anchors
    kt = d // P           # feature k-tiles
    ntil = b // nt        # n-tiles over all B columns

    f32r = mybir.dt.float32r
    f32 = mybir.dt.float32
    bf16 = mybir.dt.bfloat16
    AT = mybir.AxisListType
    OP = mybir.AluOpType

    nc = bacc.Bacc(
        "TRN2", target_bir_lowering=False, debug=False, num_devices=n_cores
    )

    embT2 = nc.dram_tensor("embT2", [d, b], f32r, kind="ExternalInput").ap()
    lhsTd = nc.dram_tensor("lhsTd", [d, r], f32r, kind="ExternalInput").ap()
    ohTd = nc.dram_tensor("ohTd", [l, b], bf16, kind="ExternalInput").ap()
    ohTCd = nc.dram_tensor("ohTCd", [l, r], bf16, kind="ExternalInput").ap()
    sqrd = nc.dram_tensor("sqrd", [2, b], bf16, kind="ExternalInput").ap()
    sqlCd = nc.dram_tensor("sqlCd", [P, mc], f32, kind="ExternalInput").ap()
    sqld = nc.dram_tensor("sqld", [P, mc], f32, kind="ExternalInput").ap()
    vldd = nc.dram_tensor("vldd", [P, mc], f32, kind="ExternalInput").ap()
    outd = nc.dram_tensor("out", [mc, 1], f32, kind="ExternalOutput").ap()

    with tile.TileContext(nc) as tc:
        for _rep in range(repeat):
            _emit_body(
                nc, tc, embT2, lhsTd, ohTd, ohTCd, sqrd, sqlCd, sqld, vldd,
                outd, b, r, mc, kt, ntil, nt, l,
            )

    nc.compile()
    return nc


def _emit_body(
    nc, tc, embT2, lhsTd, ohTd, ohTCd, sqrd, sqlCd, sqld, vldd, outd,
    b, r, mc, kt, ntil, nt, l,
):
    from contextlib import ExitStack

    import concourse.mybir as mybir

    f32r = mybir.dt.float32r
    f32 = mybir.dt.float32
    bf16 = mybir.dt.bfloat16
    AT = mybir.AxisListType
    OP = mybir.AluOpType

    if True:
        with ExitStack() as ctx:
            singles = ctx.enter_context(tc.tile_pool(name="singles", bufs=1))
            rhspool = ctx.enter_context(tc.tile_pool(name="rhspool", bufs=3))
            psum = ctx.enter_context(
                tc.tile_pool(name="psum", bufs=6, space="PSUM")
            )
            psum1 = ctx.enter_context(
                tc.tile_pool(name="psum1", bufs=1, space="PSUM")
            )
            small = ctx.enter_context(tc.tile_pool(name="small", bufs=2))

            # Resident operands
            lhs_sb = singles.tile([P, kt, r], f32r)
            lhsTr = lhsTd.rearrange("(k p) m -> k p m", p=P)
            for k in range(kt):
                nc.sync.dma_start(out=lhs_sb[:, k, :], in_=lhsTr[k])
            oh_sb = singles.tile([l, b], bf16)
            nc.sync.dma_start(out=oh_sb, in_=ohTd)
            ohc_sb = singles.tile([l, r], bf16)
            nc.sync.dma_start(out=ohc_sb, in_=ohTCd)
            sq_sb = singles.tile([2, b], bf16)
            nc.sync.dma_start(out=sq_sb, in_=sqrd)
            sql_sb = singles.tile([P, mc], f32)
            nc.sync.dma_start(out=sql_sb, in_=sqld)
            sqlC_sb = singles.tile([P, mc], f32)
            nc.sync.dma_start(out=sqlC_sb, in_=sqlCd)
            vld_sb = singles.tile([P, mc], f32)
            nc.sync.dma_start(out=vld_sb, in_=vldd)
            ones2 = singles.tile([2, P], bf16)
            nc.vector.memset(ones2, 1.0)
            onesc = singles.tile([P, 1], f32)
            nc.vector.memset(onesc, 1.0)

            # Row max / min partials per (m-chunk, n-tile)
            qmax = singles.tile([P, mc, ntil], f32)
            qmin = singles.tile([P, mc, ntil], f32)

            embT2r = embT2.rearrange("(k p) n -> k p n", p=P)
            for n in range(ntil):
                rhs = rhspool.tile([P, kt, nt], f32r, tag="rhs")
                for k in range(kt):
                    nc.sync.dma_start(
                        out=rhs[:, k, :], in_=embT2r[k, :, n * nt : (n + 1) * nt]
                    )
                for m in range(mc):
                    ps = psum.tile([P, nt], f32, tag="ps")
                    for k in range(kt):
                        nc.tensor.matmul(
                            ps,
                            lhsT=lhs_sb[:, k, m * P : (m + 1) * P],
                            rhs=rhs[:, k, :],
                            start=(k == 0),
                            stop=False,
                        )
                    nc.tensor.matmul(
                        ps,
                        lhsT=ohc_sb[:, m * P : (m + 1) * P],
                        rhs=oh_sb[:, n * nt : (n + 1) * nt],
                        start=False,
                        stop=False,
                    )
                    nc.tensor.matmul(
                        ps,
                        lhsT=ones2,
                        rhs=sq_sb[:, n * nt : (n + 1) * nt],
                        start=False,
                        stop=True,
                    )
                    nc.vector.tensor_reduce(
                        out=qmax[:, m, n : n + 1], in_=ps, axis=AT.X, op=OP.max
                    )
                    nc.vector.tensor_reduce(
                        out=qmin[:, m, n : n + 1], in_=ps, axis=AT.X, op=OP.min
                    )

            # Per-anchor loss tail
            stats = singles.tile([P, mc], f32)
            for m in range(mc):
                qmaxf = small.tile([P, 1], f32, tag="qmaxf")
                nc.vector.tensor_reduce(
                    out=qmaxf, in_=qmax[:, m, :], axis=AT.X, op=OP.max
                )
                qminf = small.tile([P, 1], f32, tag="qminf")
                nc.vector.tensor_reduce(
                    out=qminf, in_=qmin[:, m, :], axis=AT.X, op=OP.min
                )
                # dp2 = max(qmax + (sq_m - C), 0);  dn2 = max(qmin + sq_m, 0)
                dp2 = small.tile([P, 1], f32, tag="dp2")
                nc.vector.tensor_scalar(
                    out=dp2,
                    in0=qmaxf,
                    scalar1=sqlC_sb[:, m : m + 1],
                    scalar2=0.0,
                    op0=OP.add,
                    op1=OP.max,
                )
                dn2 = small.tile([P, 1], f32, tag="dn2")
                nc.vector.tensor_scalar(
                    out=dn2,
                    in0=qminf,
                    scalar1=sql_sb[:, m : m + 1],
                    scalar2=0.0,
                    op0=OP.add,
                    op1=OP.max,
                )
                dp = small.tile([P, 1], f32, tag="dp")
                nc.scalar.sqrt(dp, dp2)
                dn = small.tile([P, 1], f32, tag="dn")
                nc.scalar.sqrt(dn, dn2)
                # per = max((dp + MARGIN) - dn, 0) * valid
                pr = small.tile([P, 1], f32, tag="pr")
                nc.vector.scalar_tensor_tensor(
                    out=pr,
                    in0=dp,
                    scalar=MARGIN,
                    in1=dn,
                    op0=OP.add,
                    op1=OP.subtract,
                )
                nc.vector.tensor_scalar(
                    out=stats[:, m : m + 1],
                    in0=pr,
                    scalar1=0.0,
                    scalar2=vld_sb[:, m : m + 1],
                    op0=OP.max,
                    op1=OP.mult,
                )

            # Partition-sum each m-chunk's masked losses: out[mc,1] = stats.T @ 1
            outp = psum1.tile([mc, 1], f32)
            nc.tensor.matmul(outp, lhsT=stats, rhs=onesc, start=True, stop=True)
            out_sb = small.tile([mc, 1], f32, tag="out_sb")
            nc.vector.tensor_copy(out=out_sb, in_=outp)
            nc.sync.dma_start(out=outd, in_=out_sb)


def _get_nc(b, d, n_cores):
    key = (b, d, n_cores)
    if key not in _cache:
        _cache[key] = _build(b, d, n_cores)
    return _cache[key]


def _prep_inputs(emb, lab, n_cores):
    """Host-side sharding/layout prep. Returns (in_maps, valid_count)."""
    b, d = emb.shape
    r = b // n_cores
    mc = r // P
    bf16 = ml_dtypes.bfloat16

    embT = np.ascontiguousarray(emb.T)                       # [d, b] f32
    embT2 = np.ascontiguousarray(-2.0 * embT)                # rhs stream
    oh = (np.arange(L)[:, None] == lab[None, :])             # [L, b] bool
    ohT = oh.astype(bf16)
    ohTC = (oh.astype(np.float32) * CBIG).astype(bf16)

    sq64 = (emb.astype(np.float64) ** 2).sum(axis=1)         # [b]
    sq_hi = sq64.astype(bf16)
    sq_lo = (sq64 - sq_hi.astype(np.float64)).astype(bf16)
    sqr = np.ascontiguousarray(np.stack([sq_hi, sq_lo]))     # [2, b] bf16
    sq32 = sq64.astype(np.float32)

    counts = np.bincount(lab, minlength=L)
    valid = ((counts[lab] >= 2) & (counts[lab] <= b - 1)).astype(np.float32)

    in_maps = []
    for i in range(n_cores):
        s = slice(i * r, (i + 1) * r)
        sql = sq32[s].reshape(mc, P).T                        # [P, mc]
        vld = valid[s].reshape(mc, P).T
        in_maps.append(
            {
                "embT2": embT2,
                "lhsTd": np.ascontiguousarray(embT[:, s]),
                "ohTd": ohT,
                "ohTCd": np.ascontiguousarray(ohTC[:, s]),
                "sqrd": sqr,
                "sqlCd": np.ascontiguousarray(sql - np.float32(CBIG)),
                "sqld": np.ascontiguousarray(sql),
                "vldd": np.ascontiguousarray(vld),
            }
        )
    return in_maps, float(valid.sum())


def kernel(embeddings, labels):
    global LAST_RESULT
    from concourse.bass_utils import run_bass_kernel_spmd

    emb = np.asarray(embeddings, dtype=np.float32)
    lab = np.asarray(labels).astype(np.int64)
    b, d = emb.shape
    n_cores = NCORES

    nc = _get_nc(b, d, n_cores)
    in_maps, cnt = _prep_inputs(emb, lab, n_cores)

    res = run_bass_kernel_spmd(
        nc, in_maps, core_ids=list(range(n_cores)), trace=TRACE
    )
    LAST_RESULT = res

    total = np.float32(0.0)
    for core_out in res.results:
        total += core_out["out"].astype(np.float32).sum()
    if cnt > 0:
        loss = np.float32(total / np.float32(cnt))
    else:
        loss = np.float32(0.0)
    return np.asarray(loss, dtype=np.float32)
